# revision 3
# baseline (speedup 1.0000x reference)
"""Q8 linear (dequant matmul) on 8 Trainium2 NeuronCores — v10.

out[t, o] = sum_i (x[t, i] * scales[i]) * weight[o, i]

Tensor-parallel over out_features (14336 = 8 * 1792). Per core:
  - weights int8 [128, 32*1792] host-swizzled; DMA in tapered groups
    (small head groups for fast ramp, small tail groups for short trail),
    all on the sync HWDGE queue, issued before xs (xs rides the scalar
    queue) so the weight stream starts first.
  - int8->bf16 upcast split DVE (62%, ~0.56ns/elem 2-port mode) /
    Act (38%, ~0.86ns/elem). GpSimd unused (shared-port conflict w/ DVE).
  - ob-packed matmuls: PE column strip j computes output block j for ALL
    32 k-tiles (tile_position=(0,32j)), accumulating [32 tokens x 448]
    into psum partitions [32j:32j+32] of ONE bank. No cross-strip fold:
    host just reshapes [4,32,448] -> [32,1792].
  - HAM warmup: a run of dummy self-contained matmuls on scratch data
    keeps the PE active from the preamble until real work arrives, so
    real matmuls run at 2.4GHz instead of 1.2.
"""

import os
import sys

for _p in ("/opt/trn_rl_repo", "/root/.axon_site/_ro/trn_rl_repo"):
    if os.path.isdir(_p) and _p not in sys.path:
        sys.path.insert(0, _p)

import numpy as np
import ml_dtypes

import concourse.bass as bass
import concourse.mybir as mybir
import concourse.tile as tile
from concourse import bacc
from concourse.bass_utils import run_bass_kernel_spmd

TOKENS = 32
IN_F = 4096
OUT_F = 14336
NCORES = 8
OPC = OUT_F // NCORES  # 1792
KT = IN_F // 128  # 32
OB = 4
OBS = OPC // OB  # 448

GROUP_PLAN = (2, 2, 4, 4, 4, 4, 4, 4, 2, 1, 1)  # sum 32
DVE_SHARE = 0.63
N_WARMUP = 36

assert sum(GROUP_PLAN) == KT

_cached_nc = {}


def _emit(nc, xs, w8, out):
    gstart = []
    s = 0
    for kg in GROUP_PLAN:
        gstart.append(s)
        s += kg

    with tile.TileContext(nc) as tc:
        with (
            tc.tile_pool(name="xpool", bufs=1) as xpool,
            tc.tile_pool(name="w8pool", bufs=1) as w8pool,
            tc.tile_pool(name="wpool", bufs=1) as wpool,
            tc.tile_pool(name="opool", bufs=1) as opool,
            tc.tile_pool(name="pspool", bufs=1, space=bass.MemorySpace.PSUM) as pspool,
        ):
            # weight DMAs first on the sync queue: stream starts earliest
            w8_sbs = []
            for g, kg in enumerate(GROUP_PLAN):
                L = kg * OPC
                s0 = gstart[g] * OPC
                w8_sb = w8pool.tile([128, L], mybir.dt.int8, name=f"w8_{g}")
                nc.sync.dma_start(out=w8_sb[:], in_=w8.ap()[:, s0 : s0 + L])
                w8_sbs.append(w8_sb)

            # xs rides the scalar HWDGE queue (runs parallel to weights)
            xs_sb = xpool.tile([128, KT, TOKENS], mybir.dt.bfloat16, name="xs_sb")
            nc.scalar.dma_start(out=xs_sb[:], in_=xs.ap())

            # HAM warmup: scratch matmuls keep PE active during the ramp
            scratch = xpool.tile([128, TOKENS], mybir.dt.bfloat16, name="scratch")
            nc.gpsimd.memset(scratch[:], 0.0)
            psd = pspool.tile([TOKENS, TOKENS], mybir.dt.float32, name="psd")
            for _ in range(N_WARMUP):
                nc.tensor.matmul(
                    psd[:, :], scratch[:, :], scratch[:, :], start=True, stop=True
                )

            # casts, split DVE/Act
            w_tiles = []
            for g, kg in enumerate(GROUP_PLAN):
                L = kg * OPC
                c1 = (int(L * DVE_SHARE) // 16) * 16
                wb = wpool.tile([128, L], mybir.dt.bfloat16, name=f"wb_{g}")
                w8_sb = w8_sbs[g]
                nc.vector.tensor_copy(wb[:, 0:c1], w8_sb[:, 0:c1])
                nc.scalar.copy(wb[:, c1:L], w8_sb[:, c1:L])
                w_tiles.append(wb)

            ps = pspool.tile([128, OBS], mybir.dt.float32, name="ps")

            def gidx(ki):
                for g in range(len(GROUP_PLAN) - 1, -1, -1):
                    if gstart[g] <= ki:
                        return g, ki - gstart[g]
                raise AssertionError

            for ki in range(KT):
                g, f = gidx(ki)
                for j in range(OB):
                    nc.tensor.matmul(
                        ps[32 * j : 32 * (j + 1), :],
                        xs_sb[:, ki, :],
                        w_tiles[g][:, f * OPC + j * OBS : f * OPC + (j + 1) * OBS],
                        start=(ki == 0),
                        stop=(ki == KT - 1),
                        tile_position=(0, 32 * j),
                        skip_group_check=True,
                    )

            # tail: psum -> SBUF split by columns (DVE | Act), one out DMA
            out_sb = opool.tile([128, OBS], mybir.dt.float32, name="out_sb")
            nc.vector.tensor_copy(out_sb[:, 0:224], ps[:, 0:224])
            nc.sync.dma_start(out=out.ap()[:, 0:224], in_=out_sb[:, 0:224])
            nc.scalar.copy(out_sb[:, 224:OBS], ps[:, 224:OBS])
            nc.sync.dma_start(out=out.ap()[:, 224:OBS], in_=out_sb[:, 224:OBS])

    nc.compile()
    return nc


def _build():
    if "nc" in _cached_nc:
        return _cached_nc["nc"]
    nc = bacc.Bacc(
        "TRN2", target_bir_lowering=False, debug=False, num_devices=NCORES
    )
    xs = nc.dram_tensor(
        "xs", [128, KT * TOKENS], mybir.dt.bfloat16, kind="ExternalInput"
    )
    w8 = nc.dram_tensor("w8", [128, KT * OPC], mybir.dt.int8, kind="ExternalInput")
    out = nc.dram_tensor(
        "out", [128, OBS], mybir.dt.float32, kind="ExternalOutput"
    )
    _emit(nc, xs, w8, out)
    _cached_nc["nc"] = nc
    return nc


def make_in_maps(x, weight, scales):
    x = np.asarray(x, dtype=np.float32)
    weight = np.asarray(weight)
    scales = np.asarray(scales, dtype=np.float32)
    assert x.shape == (TOKENS, IN_F) and weight.shape == (OUT_F, IN_F)

    xs_f = x * scales[None, :]
    xs_host = np.ascontiguousarray(
        xs_f.T.reshape(KT, 128, TOKENS).transpose(1, 0, 2)
    ).astype(ml_dtypes.bfloat16).reshape(128, KT * TOKENS)

    w_all = weight.astype(np.int8)
    in_maps = []
    for c in range(NCORES):
        wc = w_all[c * OPC : (c + 1) * OPC, :]  # [OPC, IN_F]
        w_host = np.ascontiguousarray(
            wc.T.reshape(KT, 128, OPC).transpose(1, 0, 2)
        ).reshape(128, KT * OPC)
        in_maps.append({"xs": xs_host, "w8": w_host})
    return in_maps


def run(x, weight, scales, trace=False, trace_cores=None):
    nc = _build()
    in_maps = make_in_maps(x, weight, scales)
    res = run_bass_kernel_spmd(
        nc,
        in_maps,
        core_ids=list(range(NCORES)),
        trace=trace,
        trace_cores=trace_cores,
    )
    outs = []
    for c in range(NCORES):
        o = res.results[c]["out"]  # [128, OBS] = [4 strips x 32 tokens, 448]
        outs.append(o.reshape(OB, TOKENS, OBS).transpose(1, 0, 2).reshape(TOKENS, OPC))
    out = np.concatenate(outs, axis=1).astype(np.float32, copy=False)
    return out, res


def kernel(x, weight, scales):
    out, _ = run(x, weight, scales)
    return out
